# revision 11
# baseline (speedup 1.0000x reference)
"""CascadeSmoothLinear (SmoothQuant + NVFP4 block quant + GEMM + LoRA + bias) on 8 TRN2 cores.

Math per reference:
  xs  = x * smooth_scale                          [T, IN]
  x_q = nvfp4_quantize(xs)  (per-16 block amax -> scale=amax/6, snap |.|/scale
                             to levels {0,.5,1,1.5,2,3,4,6})
  out = x_q @ W^T + (xs @ A^T) @ B^T + bias       [T, OUT]

Sharding: data-parallel over tokens (B*S = 8192 -> 1024/core); weights
replicated.  W is pre-transposed on host (layout prep) so the moving GEMM
operand loads naturally.

NVFP4 snap on-device, exact (validated vs reference in fp32 emulation):
  u  = xs * (12/amax)                  (in [-12, 12])
  F  = clamp(round(u), -4, 4)          round via magic add C=1.5*2^23 (ulp-1 RNE)
  H  = 4*((u>=10) - (u<=-10))          the 4->6 level jump
  G  = clamp(round2(u-4), 0, 4) + clamp(round2(u+4), -4, 0)
       round2 = round-to-even-integer via magic add CN=-1.5*2^24 (ulp-2 RNE)
  L2 = F + G + H   (= 2*level, sign included);  x_q = L2 * (amax/12)
implemented as 3 fused custom DVE ops (8-stage budget each).
"""

import os
import sys
from contextlib import ExitStack

import numpy as np

sys.path.insert(0, "/opt/trn_rl_repo")

import concourse.bass as bass
import concourse.tile as tile
from concourse import bacc, mybir
from concourse.bass_utils import run_bass_kernel_spmd

# ---------------------------------------------------------------- custom ops
C_MAGIC = 12582912.0      # 1.5*2^23 : ulp 1 -> round-to-int
C_NEG = -25165824.0       # -1.5*2^24: ulp 2 -> round-to-even-int


def _register_custom_ops():
    import concourse.dve_ops as D
    from concourse.dve_spec import C0, C1, C2, C3, Spec, Src0, Src1, Zero
    from concourse.dve_spec import _has_src1, _spill_c3_to_src1, lower, maxx, minn
    from concourse.dve_uop import DveOpSpec

    def reg(name, spec, subdim=False):
        for op in D.OPS:
            if op.name == name:
                return op
        row = D._CUSTOM_DVE_ROW_BASE + len(D.OPS)
        shas = {}
        for ver in ("v3", "v4"):
            try:
                s = DveOpSpec(name=name, opcode=row, uops=lower(spec, ver=ver),
                              rd1_en=_has_src1(spec))
                shas[ver] = s.sha(ver)
            except Exception:
                pass
        op = D.DveOp(name, spec, subdim=subdim, uops_sha=shas)
        D.OPS.append(op)
        D._SUB_OPCODE_FOR_NAME[name] = row
        D.CUSTOM_DVE_SPECS[name] = spec
        return op

    # out = 4*((u >= 10) - (u <= -10)) + clamp(u + C, C-4, C+4)   [= H + F + C]
    # s0 = C_MAGIC, s1 = 4.0, imm2 = 10.0, in1 = [P,1] const -10.0 (C3 spill)
    fine = Spec(
        body=_spill_c3_to_src1(
            ((Src0 >= C2) - (Src0 <= C3)) * C1
            + maxx(minn(Src0 + C0, C0 + C1), C0 - C1)
        ),
        reference=lambda in0, in1, s0, s1, imm2: (
            ((in0 >= np.float32(imm2)).astype(np.float32)
             - (in0 <= in1.astype(np.float32)).astype(np.float32))
            * np.float32(s1)
            + np.maximum(np.minimum((in0 + np.float32(s0)).astype(np.float32),
                                    np.float32(s0 + s1)), np.float32(s0 - s1))
        ).astype(np.float32),
    )

    # out = clampP(u) - clampP(-u) = G   (exact magic cancellation)
    # clampP(x) = clamp(x + (CN-4), CN, CN+4)
    # s0 = CN-4, s1 = CN+4, imm2 = CN
    coarse = Spec(
        body=maxx(minn(Src0 + C0, C1), C2)
        - maxx(minn((Zero - Src0) + C0, C1), C2),
        reference=lambda in0, in1, s0, s1, imm2: (
            np.maximum(np.minimum((in0 + np.float32(s0)).astype(np.float32),
                                  np.float32(s1)), np.float32(imm2))
            - np.maximum(np.minimum(((-in0).astype(np.float32)
                                     + np.float32(s0)).astype(np.float32),
                                    np.float32(s1)), np.float32(imm2))
        ).astype(np.float32),
    )

    # out = (z + C) * sc   with sc streamed (3D, per-16-block broadcast)
    scale = Spec(
        body=(Src0 + C0) * Src1,
        reference=lambda in0, in1, s0, s1, imm2: (
            (in0 + np.float32(s0)).astype(np.float32)
            * np.asarray(in1, np.float32).reshape(in0.shape)
        ).astype(np.float32),
    )

    return (reg("NVFP4_FINE_ANT", fine), reg("NVFP4_COARSE_ANT", coarse),
            reg("NVFP4_SCALE_ANT", scale))


OP_FINE, OP_COARSE, OP_SCALE = _register_custom_ops()

# ---------------------------------------------------------------- kernel build
P = 128
IN = 4096
OUT = 4096
R = 32
BLK = 16
QF = 1024                 # free-dim sub-tile for the quantize pipeline
NQ = IN // QF             # 4
NBQ = QF // BLK           # 64 blocks per sub-tile
KCQ = QF // P             # 8 contraction chunks per sub-tile
KC = IN // P              # 32
OCW = 512                 # out-features per psum tile
NOC = OUT // OCW          # 8
F32 = mybir.dt.float32


def build_kernel(ctx: ExitStack, tc: "tile.TileContext", outs, ins, T: int,
                 groups: int = 2):
    nc = tc.nc
    x, wt, smooth, at, btb, ident = (ins["x"], ins["wt"], ins["smooth"],
                                     ins["at"], ins["btb"], ins["ident"])
    out = outs["out"]
    NT = T // P
    assert NT % groups == 0
    tpg = NT // groups

    singles = ctx.enter_context(tc.tile_pool(name="singles", bufs=1))
    xqT_pool = ctx.enter_context(tc.tile_pool(name="xqT", bufs=tpg))
    x_pool = ctx.enter_context(tc.tile_pool(name="xin", bufs=2))
    work = ctx.enter_context(tc.tile_pool(name="work", bufs=1))
    xs_pool = ctx.enter_context(tc.tile_pool(name="xs", bufs=2))
    xq_pool = ctx.enter_context(tc.tile_pool(name="xq", bufs=2))
    small = ctx.enter_context(tc.tile_pool(name="small", bufs=2))
    xsT_pool = ctx.enter_context(tc.tile_pool(name="xsT", bufs=2))
    t1_sb = ctx.enter_context(tc.tile_pool(name="t1sb", bufs=2))
    t1aug_pool = ctx.enter_context(tc.tile_pool(name="t1aug", bufs=tpg))
    w_pool = ctx.enter_context(tc.tile_pool(name="wtile", bufs=4))
    bt_pool = ctx.enter_context(tc.tile_pool(name="bt", bufs=2))
    o_pool = ctx.enter_context(tc.tile_pool(name="osb", bufs=4))
    ps_T = ctx.enter_context(tc.tile_pool(name="psT", bufs=2, space="PSUM"))
    ps_t1 = ctx.enter_context(tc.tile_pool(name="pst1", bufs=1, space="PSUM"))
    ps_t1T = ctx.enter_context(tc.tile_pool(name="pst1T", bufs=1, space="PSUM"))
    ps_out = ctx.enter_context(tc.tile_pool(name="psout", bufs=4, space="PSUM"))

    # one-time loads
    smooth_rep = singles.tile([P, IN], F32)
    nc.gpsimd.dma_start(
        out=smooth_rep,
        in_=bass.AP(tensor=smooth.tensor, offset=smooth.offset,
                    ap=[[0, P], smooth.ap[0]]),
    )
    ident_s = singles.tile([P, P], F32)
    nc.sync.dma_start(out=ident_s, in_=ident)
    cneg10 = singles.tile([P, 1], F32)
    nc.vector.memset(cneg10, -10.0)
    at_s = singles.tile([P, KC, R], F32)
    nc.sync.dma_start(out=at_s, in_=at.rearrange("(c p) r -> p c r", p=P))

    xqT_tiles = {}
    t1aug_tiles = {}

    def quantize_tile(tt):
        xqT = xqT_pool.tile([P, IN], F32)
        t1ps = ps_t1.tile([P, R], F32)
        for q in range(NQ):
            xt = x_pool.tile([P, QF], F32)
            nc.sync.dma_start(out=xt, in_=x[tt * P:(tt + 1) * P,
                                            q * QF:(q + 1) * QF])
            xs = xs_pool.tile([P, QF], F32)
            nc.vector.tensor_mul(xs, xt, smooth_rep[:, q * QF:(q + 1) * QF])
            xs3 = xs.rearrange("p (nb b) -> p nb b", b=BLK)

            amax = small.tile([P, NBQ], F32)
            nc.vector.tensor_reduce(amax, xs3, axis=mybir.AxisListType.X,
                                    op=mybir.AluOpType.max,
                                    apply_absolute_value=True)
            amaxc = small.tile([P, NBQ], F32)
            nc.vector.tensor_scalar_max(amaxc, amax, 1e-12)
            inv = small.tile([P, NBQ], F32)
            nc.vector.reciprocal(inv, amaxc)
            inv12 = small.tile([P, NBQ], F32)
            nc.vector.tensor_scalar_mul(inv12, inv, 12.0)
            sc = small.tile([P, NBQ], F32)
            nc.vector.tensor_scalar_mul(sc, amaxc, 1.0 / 12.0)

            def bcast16(t):
                return bass.AP(tensor=t.tensor, offset=t.offset,
                               ap=[t.ap[0], t.ap[1], [0, BLK]])

            u = work.tile([P, QF], F32)
            nc.vector.tensor_mul(u.rearrange("p (nb b) -> p nb b", b=BLK),
                                 xs3, bcast16(inv12))
            fa = work.tile([P, QF], F32)
            nc.vector._custom_dve(OP_FINE, out=fa, in0=u, in1=cneg10,
                                  s0=C_MAGIC, s1=4.0, imm2=10.0)
            gb = work.tile([P, QF], F32)
            nc.vector._custom_dve(OP_COARSE, out=gb, in0=u, s0=C_NEG - 4.0,
                                  s1=C_NEG + 4.0, imm2=C_NEG)
            z = work.tile([P, QF], F32)
            nc.vector.tensor_add(z, fa, gb)
            xq = xq_pool.tile([P, QF], F32)
            nc.vector._custom_dve(OP_SCALE,
                                  out=xq.rearrange("p (nb b) -> p nb b", b=BLK),
                                  in0=z.rearrange("p (nb b) -> p nb b", b=BLK),
                                  in1=bcast16(sc), s0=-C_MAGIC)

            # transpose xs (for LoRA) and xq (for main GEMM) via PE matmul
            # with identity moving operand; batch 4 chunks per PSUM bank.
            for h in range(KCQ // 4):
                psa = ps_T.tile([P, 4 * P], F32, name="psT", tag="psT")
                for j in range(4):
                    kq = h * 4 + j
                    nc.tensor.matmul(psa[:, j * P:(j + 1) * P],
                                     lhsT=xs[:, kq * P:(kq + 1) * P],
                                     rhs=ident_s, start=True, stop=True)
                xsT = xsT_pool.tile([P, 4 * P], F32)
                nc.scalar.copy(xsT, psa)
                for j in range(4):
                    kabs = q * KCQ + h * 4 + j
                    nc.tensor.matmul(t1ps,
                                     lhsT=xsT[:, j * P:(j + 1) * P],
                                     rhs=at_s[:, kabs, :],
                                     start=(kabs == 0), stop=(kabs == KC - 1))
                psb = ps_T.tile([P, 4 * P], F32, name="psT", tag="psT")
                for j in range(4):
                    kq = h * 4 + j
                    nc.tensor.matmul(psb[:, j * P:(j + 1) * P],
                                     lhsT=xq[:, kq * P:(kq + 1) * P],
                                     rhs=ident_s, start=True, stop=True)
                nc.scalar.copy(
                    xqT[:, q * QF + h * 4 * P:q * QF + (h + 1) * 4 * P], psb)

        t1s = t1_sb.tile([P, R], F32)
        nc.scalar.copy(t1s, t1ps)
        t1Tps = ps_t1T.tile([R, P], F32)
        nc.tensor.matmul(t1Tps, lhsT=t1s, rhs=ident_s, start=True,
                         stop=True)
        t1aug = t1aug_pool.tile([R + 1, P], F32)
        nc.scalar.copy(t1aug[0:R, :], t1Tps)
        nc.vector.memset(t1aug[R:R + 1, :], 1.0)
        xqT_tiles[tt] = xqT
        t1aug_tiles[tt] = t1aug

    for g in range(groups):
        tts = range(g * tpg, (g + 1) * tpg)
        for tt in tts:
            quantize_tile(tt)
        for oc in range(NOC):
            btoc = bt_pool.tile([R + 1, OCW], F32)
            nc.sync.dma_start(out=btoc, in_=btb[:, oc * OCW:(oc + 1) * OCW])
            pso = {}
            for k in range(KC):
                wtt = w_pool.tile([P, OCW], F32)
                nc.sync.dma_start(out=wtt, in_=wt[k * P:(k + 1) * P,
                                                  oc * OCW:(oc + 1) * OCW])
                for tt in tts:
                    if k == 0:
                        pso[tt] = ps_out.tile([P, OCW], F32, name="psout", tag="psout")
                    nc.tensor.matmul(pso[tt],
                                     lhsT=xqT_tiles[tt][:, k * P:(k + 1) * P],
                                     rhs=wtt, start=(k == 0), stop=False)
            for tt in tts:
                nc.tensor.matmul(pso[tt], lhsT=t1aug_tiles[tt], rhs=btoc,
                                 start=False, stop=True)
                osb = o_pool.tile([P, OCW], F32)
                nc.scalar.copy(osb, pso[tt])
                nc.sync.dma_start(out=out[tt * P:(tt + 1) * P,
                                          oc * OCW:(oc + 1) * OCW], in_=osb)


def build_module(T: int, groups: int = 2, num_devices: int = 1):
    nc = bacc.Bacc("TRN2", target_bir_lowering=False, debug=False,
                   enable_asserts=False, num_devices=num_devices)
    ins = {
        "x": nc.dram_tensor("x", [T, IN], F32, kind="ExternalInput").ap(),
        "wt": nc.dram_tensor("wt", [IN, OUT], F32, kind="ExternalInput").ap(),
        "smooth": nc.dram_tensor("smooth", [IN], F32,
                                 kind="ExternalInput").ap(),
        "at": nc.dram_tensor("at", [IN, R], F32, kind="ExternalInput").ap(),
        "btb": nc.dram_tensor("btb", [R + 1, OUT], F32,
                              kind="ExternalInput").ap(),
        "ident": nc.dram_tensor("ident", [P, P], F32,
                                kind="ExternalInput").ap(),
    }
    outs = {"out": nc.dram_tensor("out", [T, OUT], F32,
                                  kind="ExternalOutput").ap()}
    with tile.TileContext(nc) as tc:
        with ExitStack() as ctx:
            build_kernel(ctx, tc, outs, ins, T=T, groups=groups)
    nc.compile()
    return nc


_NC_CACHE = {}


def kernel(x, smooth_scale, w_quantized, lora_a, lora_b, bias):
    n_cores = 8
    xf = np.ascontiguousarray(np.asarray(x, dtype=np.float32).reshape(-1, IN))
    tokens = xf.shape[0]
    T = tokens // n_cores
    wt = np.ascontiguousarray(np.asarray(w_quantized, dtype=np.float32).T)
    smooth = np.ascontiguousarray(np.asarray(smooth_scale, dtype=np.float32))
    at = np.ascontiguousarray(np.asarray(lora_a, dtype=np.float32).T)
    btb = np.ascontiguousarray(np.concatenate(
        [np.asarray(lora_b, dtype=np.float32).T,
         np.asarray(bias, dtype=np.float32)[None, :]], axis=0))
    ident = np.eye(P, dtype=np.float32)

    key = (T,)
    if key not in _NC_CACHE:
        _NC_CACHE[key] = build_module(T, groups=2, num_devices=1)
    nc = _NC_CACHE[key]

    in_maps = [
        {"x": np.ascontiguousarray(xf[c * T:(c + 1) * T]), "wt": wt,
         "smooth": smooth, "at": at, "btb": btb, "ident": ident}
        for c in range(n_cores)
    ]
    res = run_bass_kernel_spmd(nc, in_maps, core_ids=list(range(n_cores)))
    out = np.concatenate([r["out"] for r in res.results], axis=0)
    return out.reshape(np.asarray(x).shape[:-1] + (OUT,))


# revision 12
# speedup vs baseline: 1924.6549x; 1924.6549x over previous
"""CascadeSmoothLinear (SmoothQuant + NVFP4 block quant + GEMM + LoRA + bias) on 8 TRN2 cores.

Math per reference:
  xs  = x * smooth_scale                          [T, IN]
  x_q = nvfp4_quantize(xs)  (per-16 block amax -> scale=amax/6, snap |.|/scale
                             to levels {0,.5,1,1.5,2,3,4,6})
  out = x_q @ W^T + (xs @ A^T) @ B^T + bias       [T, OUT]

Sharding: data-parallel over tokens (B*S = 8192 -> 1024/core); weights
replicated.  W is pre-transposed on host (layout prep) so the moving GEMM
operand loads naturally.

NVFP4 snap on-device, exact (validated vs reference in fp32 emulation):
  u  = xs * (12/amax)                  (in [-12, 12])
  F  = clamp(round(u), -4, 4)          round via magic add C=1.5*2^23 (ulp-1 RNE)
  H  = 4*((u>=10) - (u<=-10))          the 4->6 level jump
  G  = clamp(round2(u-4), 0, 4) + clamp(round2(u+4), -4, 0)
       round2 = round-to-even-integer via magic add CN=-1.5*2^24 (ulp-2 RNE)
  L2 = F + G + H   (= 2*level, sign included);  x_q = L2 * (amax/12)
implemented as 3 fused custom DVE ops (8-stage budget each).
"""

import os
import sys
from contextlib import ExitStack

import numpy as np

sys.path.insert(0, "/opt/trn_rl_repo")

import concourse.bass as bass
import concourse.tile as tile
from concourse import bacc, mybir
from concourse.bass_utils import run_bass_kernel_spmd

# ---------------------------------------------------------------- custom ops
C_MAGIC = 12582912.0      # 1.5*2^23 : ulp 1 -> round-to-int
C_NEG = -25165824.0       # -1.5*2^24: ulp 2 -> round-to-even-int


def _register_custom_ops():
    import concourse.dve_ops as D
    from concourse.dve_spec import C0, C1, C2, C3, Spec, Src0, Src1, Zero
    from concourse.dve_spec import _has_src1, _spill_c3_to_src1, lower, maxx, minn
    from concourse.dve_uop import DveOpSpec

    def reg(name, spec, subdim=False):
        for op in D.OPS:
            if op.name == name:
                return op
        row = D._CUSTOM_DVE_ROW_BASE + len(D.OPS)
        shas = {}
        for ver in ("v3", "v4"):
            try:
                s = DveOpSpec(name=name, opcode=row, uops=lower(spec, ver=ver),
                              rd1_en=_has_src1(spec))
                shas[ver] = s.sha(ver)
            except Exception:
                pass
        op = D.DveOp(name, spec, subdim=subdim, uops_sha=shas)
        D.OPS.append(op)
        D._SUB_OPCODE_FOR_NAME[name] = row
        D.CUSTOM_DVE_SPECS[name] = spec
        return op

    # out = 4*((u >= 10) - (u <= -10)) + clamp(u + C, C-4, C+4)   [= H + F + C]
    # s0 = C_MAGIC, s1 = 4.0, imm2 = 10.0, in1 = [P,1] const -10.0 (C3 spill)
    fine = Spec(
        body=_spill_c3_to_src1(
            ((Src0 > C2) - (Src0 < C3)) * C1
            + maxx(minn(Src0 + C0, C0 + C1), C0 - C1)
        ),
        reference=lambda in0, in1, s0, s1, imm2: (
            ((in0 > np.float32(imm2)).astype(np.float32)
             - (in0 < in1.astype(np.float32)).astype(np.float32))
            * np.float32(s1)
            + np.maximum(np.minimum((in0 + np.float32(s0)).astype(np.float32),
                                    np.float32(s0 + s1)), np.float32(s0 - s1))
        ).astype(np.float32),
    )

    # out = clampP(u) - clampP(-u) = G   (exact magic cancellation)
    # clampP(x) = clamp(x + (CN-4), CN, CN+4)
    # s0 = CN-4, s1 = CN+4, imm2 = CN
    coarse = Spec(
        body=maxx(minn(Src0 + C0, C1), C2)
        - maxx(minn((Zero - Src0) + C0, C1), C2),
        reference=lambda in0, in1, s0, s1, imm2: (
            np.maximum(np.minimum((in0 + np.float32(s0)).astype(np.float32),
                                  np.float32(s1)), np.float32(imm2))
            - np.maximum(np.minimum(((-in0).astype(np.float32)
                                     + np.float32(s0)).astype(np.float32),
                                    np.float32(s1)), np.float32(imm2))
        ).astype(np.float32),
    )

    # out = (z + C) * sc   with sc streamed (3D, per-16-block broadcast)
    scale = Spec(
        body=(Src0 + C0) * Src1,
        reference=lambda in0, in1, s0, s1, imm2: (
            (in0 + np.float32(s0)).astype(np.float32)
            * np.asarray(in1, np.float32).reshape(in0.shape)
        ).astype(np.float32),
    )

    return (reg("NVFP4_FINE_ANT", fine), reg("NVFP4_COARSE_ANT", coarse),
            reg("NVFP4_SCALE_ANT", scale))


OP_FINE, OP_COARSE, OP_SCALE = _register_custom_ops()

# ---------------------------------------------------------------- kernel build
P = 128
IN = 4096
OUT = 4096
R = 32
BLK = 16
QF = 1024                 # free-dim sub-tile for the quantize pipeline
NQ = IN // QF             # 4
NBQ = QF // BLK           # 64 blocks per sub-tile
KCQ = QF // P             # 8 contraction chunks per sub-tile
KC = IN // P              # 32
OCW = 512                 # out-features per psum tile
NOC = OUT // OCW          # 8
F32 = mybir.dt.float32


def build_kernel(ctx: ExitStack, tc: "tile.TileContext", outs, ins, T: int,
                 groups: int = 2):
    nc = tc.nc
    x, wt, smooth, at, btb, ident = (ins["x"], ins["wt"], ins["smooth"],
                                     ins["at"], ins["btb"], ins["ident"])
    out = outs["out"]
    NT = T // P
    assert NT % groups == 0
    tpg = NT // groups

    singles = ctx.enter_context(tc.tile_pool(name="singles", bufs=1))
    xqT_pool = ctx.enter_context(tc.tile_pool(name="xqT", bufs=tpg))
    x_pool = ctx.enter_context(tc.tile_pool(name="xin", bufs=2))
    work = ctx.enter_context(tc.tile_pool(name="work", bufs=1))
    xs_pool = ctx.enter_context(tc.tile_pool(name="xs", bufs=2))
    xq_pool = ctx.enter_context(tc.tile_pool(name="xq", bufs=2))
    small = ctx.enter_context(tc.tile_pool(name="small", bufs=2))
    xsT_pool = ctx.enter_context(tc.tile_pool(name="xsT", bufs=2))
    t1_sb = ctx.enter_context(tc.tile_pool(name="t1sb", bufs=2))
    t1aug_pool = ctx.enter_context(tc.tile_pool(name="t1aug", bufs=tpg))
    w_pool = ctx.enter_context(tc.tile_pool(name="wtile", bufs=4))
    bt_pool = ctx.enter_context(tc.tile_pool(name="bt", bufs=2))
    o_pool = ctx.enter_context(tc.tile_pool(name="osb", bufs=4))
    ps_T = ctx.enter_context(tc.tile_pool(name="psT", bufs=2, space="PSUM"))
    ps_t1 = ctx.enter_context(tc.tile_pool(name="pst1", bufs=1, space="PSUM"))
    ps_t1T = ctx.enter_context(tc.tile_pool(name="pst1T", bufs=1, space="PSUM"))
    ps_out = ctx.enter_context(tc.tile_pool(name="psout", bufs=4, space="PSUM"))

    # one-time loads
    smooth_rep = singles.tile([P, IN], F32)
    nc.gpsimd.dma_start(
        out=smooth_rep,
        in_=bass.AP(tensor=smooth.tensor, offset=smooth.offset,
                    ap=[[0, P], smooth.ap[0]]),
    )
    ident_s = singles.tile([P, P], F32)
    nc.sync.dma_start(out=ident_s, in_=ident)
    cneg10 = singles.tile([P, 1], F32)
    nc.vector.memset(cneg10, -10.0)
    at_s = singles.tile([P, KC, R], F32)
    nc.sync.dma_start(out=at_s, in_=at.rearrange("(c p) r -> p c r", p=P))

    xqT_tiles = {}
    t1aug_tiles = {}

    def quantize_tile(tt):
        xqT = xqT_pool.tile([P, IN], F32)
        t1ps = ps_t1.tile([P, R], F32)
        for q in range(NQ):
            xt = x_pool.tile([P, QF], F32)
            nc.sync.dma_start(out=xt, in_=x[tt * P:(tt + 1) * P,
                                            q * QF:(q + 1) * QF])
            xs = xs_pool.tile([P, QF], F32)
            nc.vector.tensor_mul(xs, xt, smooth_rep[:, q * QF:(q + 1) * QF])
            xs3 = xs.rearrange("p (nb b) -> p nb b", b=BLK)

            amax = small.tile([P, NBQ], F32)
            nc.vector.tensor_reduce(amax, xs3, axis=mybir.AxisListType.X,
                                    op=mybir.AluOpType.max,
                                    apply_absolute_value=True)
            amaxc = small.tile([P, NBQ], F32)
            nc.vector.tensor_scalar_max(amaxc, amax, 1e-12)
            inv = small.tile([P, NBQ], F32)
            nc.vector.reciprocal(inv, amaxc)
            inv12 = small.tile([P, NBQ], F32)
            nc.vector.tensor_scalar_mul(inv12, inv, 12.0)
            sc = small.tile([P, NBQ], F32)
            nc.vector.tensor_scalar_mul(sc, amaxc, 1.0 / 12.0)

            def bcast16(t):
                return bass.AP(tensor=t.tensor, offset=t.offset,
                               ap=[t.ap[0], t.ap[1], [0, BLK]])

            u = work.tile([P, QF], F32)
            nc.vector.tensor_mul(u.rearrange("p (nb b) -> p nb b", b=BLK),
                                 xs3, bcast16(inv12))
            fa = work.tile([P, QF], F32)
            nc.vector._custom_dve(OP_FINE, out=fa, in0=u, in1=cneg10,
                                  s0=C_MAGIC, s1=4.0, imm2=10.0)
            gb = work.tile([P, QF], F32)
            nc.vector._custom_dve(OP_COARSE, out=gb, in0=u, s0=C_NEG - 4.0,
                                  s1=C_NEG + 4.0, imm2=C_NEG)
            z = work.tile([P, QF], F32)
            nc.vector.tensor_add(z, fa, gb)
            xq = xq_pool.tile([P, QF], F32)
            nc.vector._custom_dve(OP_SCALE,
                                  out=xq.rearrange("p (nb b) -> p nb b", b=BLK),
                                  in0=z.rearrange("p (nb b) -> p nb b", b=BLK),
                                  in1=bcast16(sc), s0=-C_MAGIC)

            # transpose xs (for LoRA) and xq (for main GEMM) via PE matmul
            # with identity moving operand; batch 4 chunks per PSUM bank.
            for h in range(KCQ // 4):
                psa = ps_T.tile([P, 4 * P], F32, name="psT", tag="psT")
                for j in range(4):
                    kq = h * 4 + j
                    nc.tensor.matmul(psa[:, j * P:(j + 1) * P],
                                     lhsT=xs[:, kq * P:(kq + 1) * P],
                                     rhs=ident_s, start=True, stop=True)
                xsT = xsT_pool.tile([P, 4 * P], F32)
                nc.scalar.copy(xsT, psa)
                for j in range(4):
                    kabs = q * KCQ + h * 4 + j
                    nc.tensor.matmul(t1ps,
                                     lhsT=xsT[:, j * P:(j + 1) * P],
                                     rhs=at_s[:, kabs, :],
                                     start=(kabs == 0), stop=(kabs == KC - 1))
                psb = ps_T.tile([P, 4 * P], F32, name="psT", tag="psT")
                for j in range(4):
                    kq = h * 4 + j
                    nc.tensor.matmul(psb[:, j * P:(j + 1) * P],
                                     lhsT=xq[:, kq * P:(kq + 1) * P],
                                     rhs=ident_s, start=True, stop=True)
                nc.scalar.copy(
                    xqT[:, q * QF + h * 4 * P:q * QF + (h + 1) * 4 * P], psb)

        t1s = t1_sb.tile([P, R], F32)
        nc.scalar.copy(t1s, t1ps)
        t1Tps = ps_t1T.tile([R, P], F32)
        nc.tensor.matmul(t1Tps, lhsT=t1s, rhs=ident_s, start=True,
                         stop=True)
        t1aug = t1aug_pool.tile([R + 1, P], F32)
        nc.scalar.copy(t1aug[0:R, :], t1Tps)
        nc.vector.memset(t1aug[R:R + 1, :], 1.0)
        xqT_tiles[tt] = xqT
        t1aug_tiles[tt] = t1aug

    for g in range(groups):
        tts = range(g * tpg, (g + 1) * tpg)
        for tt in tts:
            quantize_tile(tt)
        for oc in range(NOC):
            btoc = bt_pool.tile([R + 1, OCW], F32)
            nc.sync.dma_start(out=btoc, in_=btb[:, oc * OCW:(oc + 1) * OCW])
            pso = {}
            for k in range(KC):
                wtt = w_pool.tile([P, OCW], F32)
                nc.sync.dma_start(out=wtt, in_=wt[k * P:(k + 1) * P,
                                                  oc * OCW:(oc + 1) * OCW])
                for tt in tts:
                    if k == 0:
                        pso[tt] = ps_out.tile([P, OCW], F32, name="psout", tag="psout")
                    nc.tensor.matmul(pso[tt],
                                     lhsT=xqT_tiles[tt][:, k * P:(k + 1) * P],
                                     rhs=wtt, start=(k == 0), stop=False)
            for tt in tts:
                nc.tensor.matmul(pso[tt], lhsT=t1aug_tiles[tt], rhs=btoc,
                                 start=False, stop=True)
                osb = o_pool.tile([P, OCW], F32)
                nc.scalar.copy(osb, pso[tt])
                nc.sync.dma_start(out=out[tt * P:(tt + 1) * P,
                                          oc * OCW:(oc + 1) * OCW], in_=osb)


def build_module(T: int, groups: int = 2, num_devices: int = 1):
    nc = bacc.Bacc("TRN2", target_bir_lowering=False, debug=False,
                   enable_asserts=False, num_devices=num_devices)
    ins = {
        "x": nc.dram_tensor("x", [T, IN], F32, kind="ExternalInput").ap(),
        "wt": nc.dram_tensor("wt", [IN, OUT], F32, kind="ExternalInput").ap(),
        "smooth": nc.dram_tensor("smooth", [IN], F32,
                                 kind="ExternalInput").ap(),
        "at": nc.dram_tensor("at", [IN, R], F32, kind="ExternalInput").ap(),
        "btb": nc.dram_tensor("btb", [R + 1, OUT], F32,
                              kind="ExternalInput").ap(),
        "ident": nc.dram_tensor("ident", [P, P], F32,
                                kind="ExternalInput").ap(),
    }
    outs = {"out": nc.dram_tensor("out", [T, OUT], F32,
                                  kind="ExternalOutput").ap()}
    with tile.TileContext(nc) as tc:
        with ExitStack() as ctx:
            build_kernel(ctx, tc, outs, ins, T=T, groups=groups)
    nc.compile()
    return nc


_NC_CACHE = {}


def kernel(x, smooth_scale, w_quantized, lora_a, lora_b, bias):
    n_cores = 8
    xf = np.ascontiguousarray(np.asarray(x, dtype=np.float32).reshape(-1, IN))
    tokens = xf.shape[0]
    T = tokens // n_cores
    wt = np.ascontiguousarray(np.asarray(w_quantized, dtype=np.float32).T)
    smooth = np.ascontiguousarray(np.asarray(smooth_scale, dtype=np.float32))
    at = np.ascontiguousarray(np.asarray(lora_a, dtype=np.float32).T)
    btb = np.ascontiguousarray(np.concatenate(
        [np.asarray(lora_b, dtype=np.float32).T,
         np.asarray(bias, dtype=np.float32)[None, :]], axis=0))
    ident = np.eye(P, dtype=np.float32)

    key = (T,)
    if key not in _NC_CACHE:
        _NC_CACHE[key] = build_module(T, groups=2, num_devices=1)
    nc = _NC_CACHE[key]

    in_maps = [
        {"x": np.ascontiguousarray(xf[c * T:(c + 1) * T]), "wt": wt,
         "smooth": smooth, "at": at, "btb": btb, "ident": ident}
        for c in range(n_cores)
    ]
    res = run_bass_kernel_spmd(nc, in_maps, core_ids=list(range(n_cores)))
    out = np.concatenate([r["out"] for r in res.results], axis=0)
    return out.reshape(np.asarray(x).shape[:-1] + (OUT,))
